# revision 8
# baseline (speedup 1.0000x reference)
"""Trainium2 Bass kernel for nn_CNN_NCDE_Model (CNN -> channel attention ->
natural-cubic-spline NCDE integrated with fixed-step RK4).

Strategy: pure data parallelism over batch (64 -> 8 cores x 8 images).
The spline coefficient solve + derivative evaluation collapses into one
constant matrix H[253,64] applied to seq (host-precomputed from the
tridiagonal system; data-independent), so the whole pre-ODE stage is a
small set of matmuls. The ODE scan (126 RK4 steps = 504 vector-field
evals) is the dominant cost: each eval is [8,64]@f1 -> relu ->
[8,128]@f2(32768x128, bf16) -> +bias -> tanh -> contraction with dX.
f2 weights stay resident in SBUF (bf16) and are streamed through the PE
as 256 stationary tiles per eval; the dX contraction runs on the PE as
per-batch M=1 accumulated matmuls, fully avoiding cross-partition
reductions.
"""
import numpy as np
import ml_dtypes

import concourse.bacc as bacc
import concourse.bass as bass
import concourse.mybir as mybir
import concourse.tile as tile
from concourse.bass_utils import run_bass_kernel_spmd

F32 = mybir.dt.float32
BF16 = mybir.dt.bfloat16
AF = mybir.ActivationFunctionType
ALU = mybir.AluOpType

N_CORES = 8
BPC = 8            # batch per core
L = 64             # sequence length after pooling
NQ = 253           # quarter-time points t=q/4, q=0..252
NSTEPS = 126
DT = 0.5


def _make_H():
    """H[q,l] with dX(t_q)[b,c] = sum_l H[q,l]*seq[b,l,c] (natural cubic)."""
    n = L - 2
    A = 4.0 * np.eye(n) + np.eye(n, k=1) + np.eye(n, k=-1)
    Ainv = np.linalg.inv(A)
    R = np.zeros((n, L))
    for j in range(n):
        R[j, j] += 6.0
        R[j, j + 1] += -12.0
        R[j, j + 2] += 6.0
    Mmat = np.zeros((L, L))
    Mmat[1:L - 1, :] = Ainv @ R
    H = np.zeros((NQ, L))
    for q in range(NQ):
        seg = min(q // 4, L - 2)
        fr = q / 4.0 - seg
        al = -1.0 / 3.0 + fr - fr * fr / 2.0
        be = -1.0 / 6.0 + fr * fr / 2.0
        H[q, seg] += -1.0
        H[q, seg + 1] += 1.0
        H[q, :] += al * Mmat[seg, :] + be * Mmat[seg + 1, :]
    return H.astype(np.float32)


def _ap(t_ap, offset, dims):
    return bass.AP(t_ap.tensor, offset, [list(d) for d in dims])


def _build(phase=99, nsteps=NSTEPS, debug_dump=False, unroll=False, timing_mode=False, relu_dve=True, split_ctr=True, abl_ndt=4, abl_nb=BPC, abl_no_act=False):
    nc = bacc.Bacc("TRN2", target_bir_lowering=False, debug=True)

    def din(name, shape, dt):
        return nc.dram_tensor(name, shape, dt, kind="ExternalInput")

    x_pad = din("x_pad", [36, 8 * 132], BF16)      # padded input, h x (img,w)
    w1col = din("w1col", [25, 32], BF16)           # conv1 as K=25 lhsT
    c1b = din("c1b", [32, 1], F32)
    w2taps = din("w2taps", [32, 9 * 32], BF16)     # conv2 per-tap lhsT
    c2b = din("c2b", [32, 1], F32)
    a1w = din("a1w", [32, 4], F32)                 # att fc1 lhsT (pre-scaled /1024)
    a1b = din("a1b", [4, 1], F32)
    a2w = din("a2w", [4, 32], F32)
    a2b = din("a2b", [32, 1], F32)
    HTd = din("HT", [64, NQ], F32)                 # H^T
    iwT = din("iwT", [128, 4 * 64], F32)           # initial_w^T tiles
    ibd = din("ib", [64, 1], F32)
    w1T = din("w1T", [64, 128], BF16)              # f1_w^T
    f1bd = din("f1b", [128, 1], F32)
    w2T = din("w2T", [128, 32768], BF16)           # f2_w^T
    b2r = din("b2r", [128, 4 * 64], F32)           # f2_b as [c, (dt,h)]
    owT = din("owT", [64, 2], F32)
    obd = din("ob", [2, 1], F32)
    idmd = din("idm", [32, 32], F32)
    out_d = nc.dram_tensor("out", [BPC, 2], F32, kind="ExternalOutput")
    if debug_dump:
        dbg_pooled = nc.dram_tensor("dbg_pooled", [32, 8192], F32, kind="ExternalOutput")
        dbg_p2T = nc.dram_tensor("dbg_p2T", [64, 4096], F32, kind="ExternalOutput")
        dbg_s0 = nc.dram_tensor("dbg_s0", [128, 32], F32, kind="ExternalOutput")
        dbg_dx = nc.dram_tensor("dbg_dx", [128, NQ * 32], mybir.dt.bfloat16, kind="ExternalOutput")
        dbg_z0 = nc.dram_tensor("dbg_z0", [64, 8], F32, kind="ExternalOutput")
        dbg_k = nc.dram_tensor("dbg_k", [64, 8 * 4], F32, kind="ExternalOutput")
        dbg_u2 = nc.dram_tensor("dbg_u2", [128, 512], mybir.dt.bfloat16, kind="ExternalOutput")
        dbg_u = nc.dram_tensor("dbg_u", [128, 8], mybir.dt.bfloat16, kind="ExternalOutput")
        dbg_zf = nc.dram_tensor("dbg_zf", [64, 8], F32, kind="ExternalOutput")

    with tile.TileContext(nc) as tc:
        cpool = tc.tile_pool(name="consts", bufs=1)
        cp = cpool.__enter__()

        def load_const(dram, shape, dt):
            t = cp.tile(shape, dt, tag=f"c_{dram.name}")
            nc.gpsimd.dma_start(t[:], dram[:])
            return t

        w1col_s = load_const(w1col, [25, 32], BF16)
        c1b_s = load_const(c1b, [32, 1], F32)
        w2taps_s = load_const(w2taps, [32, 288], BF16)
        c2b_s = load_const(c2b, [32, 1], F32)
        a1w_s = load_const(a1w, [32, 4], F32)
        a1b_s = load_const(a1b, [4, 1], F32)
        a2w_s = load_const(a2w, [4, 32], F32)
        a2b_s = load_const(a2b, [32, 1], F32)
        HT_s = load_const(HTd, [64, NQ], F32)
        iwT_s = load_const(iwT, [128, 256], F32)
        ib_s = load_const(ibd, [64, 1], F32)
        w1T_s = load_const(w1T, [64, 128], BF16)
        f1b_s = load_const(f1bd, [128, 1], F32)
        b2r_s = load_const(b2r, [128, 256], F32)
        owT_s = load_const(owT, [64, 2], F32)
        ob_s = load_const(obd, [2, 1], F32)
        idm_s = load_const(idmd, [32, 32], F32)
        pooled = cp.tile([32, 8192], F32)
        pooled_r = pooled[:].rearrange("p (i hp w) -> p i hp w", i=8, hp=16, w=64)

        # ---------------- CNN ----------------
        if phase >= 1:
          with tc.tile_pool(name="cnn", bufs=1) as cnn, \
             tc.tile_pool(name="cnn2", bufs=2) as cnn2, \
             tc.tile_pool(name="cnnps", bufs=2, space="PSUM") as cnnps:
            c1pad = cnn.tile([32, 8 * 34 * 130], BF16)
            nc.gpsimd.memset(c1pad[:], 0.0)
            c1pad_r = c1pad[:].rearrange("p (i h w) -> p i h w", i=8, h=34, w=130)

            # conv1, processed in 4 chunks of 8 output rows
            for hc in range(4):
                h0 = hc * 8
                imcol = cnn2.tile([25, 8192], BF16, tag="imcol")
                for dy in range(5):
                    src = _ap(x_pad[:], (h0 + dy) * 1056,
                              [(1, 5), (1056, 8), (132, 8), (1, 128)])
                    nc.gpsimd.dma_start(imcol[dy * 5:(dy + 1) * 5, :], src)
                for c in range(16):
                    h = h0 + c // 2
                    ihalf = c % 2
                    ps = cnnps.tile([32, 512], F32, tag="c1")
                    nc.tensor.matmul(ps[:], w1col_s[:], imcol[:, c * 512:(c + 1) * 512],
                                     start=True, stop=True)
                    dest = c1pad_r[:, 4 * ihalf:4 * ihalf + 4, 1 + h, 1:129]
                    nc.scalar.activation(dest, ps[:].rearrange("p (i w) -> p i w", i=4),
                                         AF.Relu, bias=c1b_s[:, 0:1])

            # conv2 (tap-accumulated) + relu + maxpool, per image / 4-row chunk
            for img in range(8):
                for hc in range(8):
                    h0 = hc * 4
                    ps2 = cnnps.tile([32, 512], F32, tag="c2")
                    for tap in range(9):
                        dy, dx = tap // 3, tap % 3
                        rhs = c1pad_r[:, img, h0 + dy:h0 + dy + 4, dx:dx + 128]
                        nc.tensor.matmul(ps2[:], w2taps_s[:, tap * 32:(tap + 1) * 32],
                                         rhs, start=(tap == 0), stop=(tap == 8))
                    c2c = cnn2.tile([32, 512], F32, tag="c2out")
                    nc.scalar.activation(c2c[:], ps2[:], AF.Relu, bias=c2b_s[:, 0:1])
                    c2r = c2c[:].rearrange("p (h a w b) -> p h a w b", h=2, a=2, w=64, b=2)
                    t1 = cnn2.tile([32, 128], F32, tag="pa")
                    t1r = t1[:].rearrange("p (h w) -> p h w", h=2)
                    t2 = cnn2.tile([32, 128], F32, tag="pb")
                    t2r = t2[:].rearrange("p (h w) -> p h w", h=2)
                    nc.vector.tensor_tensor(t1r, c2r[:, :, 0, :, 0], c2r[:, :, 0, :, 1], op=ALU.max)
                    nc.vector.tensor_tensor(t2r, c2r[:, :, 1, :, 0], c2r[:, :, 1, :, 1], op=ALU.max)
                    dest = pooled_r[:, img, h0 // 2:h0 // 2 + 2, :]
                    nc.vector.tensor_tensor(dest, t1r, t2r, op=ALU.max)

        # ---------------- attention ----------------
        if phase >= 2:
          with tc.tile_pool(name="att", bufs=1) as att, \
             tc.tile_pool(name="attps", bufs=1, space="PSUM") as attps:
            satt = att.tile([32, 8], F32)
            nc.vector.tensor_reduce(satt[:], pooled[:].rearrange("p (i f) -> p i f", i=8),
                                    axis=mybir.AxisListType.X, op=ALU.add)
            a1ps = attps.tile([4, 8], F32, tag="a1")
            nc.tensor.matmul(a1ps[:], a1w_s[:], satt[:], start=True, stop=True)
            att1 = att.tile([4, 8], F32)
            nc.scalar.activation(att1[:], a1ps[:], AF.Relu, bias=a1b_s[:, 0:1])
            a2ps = attps.tile([32, 8], F32, tag="a2")
            nc.tensor.matmul(a2ps[:], a2w_s[:], att1[:], start=True, stop=True)
            attw = att.tile([32, 8], F32)
            nc.scalar.activation(attw[:], a2ps[:], AF.Sigmoid, bias=a2b_s[:, 0:1])
            nc.vector.tensor_tensor(
                pooled[:].rearrange("p (i f) -> p i f", i=8),
                pooled[:].rearrange("p (i f) -> p i f", i=8),
                attw[:].unsqueeze(-1).broadcast_to((32, 8, 1024)),
                op=ALU.mult)

        # ---------------- spline/dX table + z0 + ODE ----------------
        if phase >= 3:
          with tc.tile_pool(name="ode", bufs=1) as ode, \
             tc.tile_pool(name="seqp", bufs=2) as seqp, \
             tc.tile_pool(name="stg", bufs=2) as stg, \
             tc.tile_pool(name="u2p", bufs=5) as u2p:

            w2sb = ode.tile([128, 32768], BF16)
            for ch in range(8):
                nc.gpsimd.dma_start(w2sb[:, ch * 4096:(ch + 1) * 4096],
                                    w2T[:, ch * 4096:(ch + 1) * 4096])
            dxtab = ode.tile([128, NQ * 32], BF16)   # [c, (q, dt, b)]
            dxtab_r = dxtab[:].rearrange("p (q c b) -> p q c b", q=NQ, c=4, b=8)

            p2T = ode.tile([64, 8 * 512], F32)   # seq, [w][img][oc*16+hp]
            p2T_r = p2T[:].rearrange("w (i o h) -> w i o h", i=8, o=32, h=16)
            with tc.tile_pool(name="dxps", bufs=2, space="PSUM") as dxps:
                for img in range(8):
                    for hp in range(16):
                        tp = dxps.tile([64, 32], F32, tag="tp")
                        nc.tensor.transpose(tp[:], pooled_r[:, img, hp, :], idm_s[:, :])
                        nc.scalar.copy(p2T_r[:, img, :, hp], tp[:])
                for b in range(BPC):
                    for ct in range(4):
                        dps = dxps.tile([128, NQ], F32, tag="dx")
                        nc.tensor.matmul(dps[:], p2T[:, b * 512 + ct * 128:b * 512 + (ct + 1) * 128],
                                         HT_s[:], start=True, stop=True)
                        nc.scalar.copy(dxtab_r[:, :, ct, b], dps[:])
                s0 = ode.tile([128, 32], F32)
                for b in range(BPC):
                    for ct in range(4):
                        sp = dxps.tile([128, 1], F32, tag="s0p")
                        nc.tensor.transpose(
                            sp[:], p2T[0:1, b * 512 + ct * 128:b * 512 + (ct + 1) * 128],
                            idm_s[0:1, 0:1])
                        nc.scalar.copy(s0[:, ct * 8 + b:ct * 8 + b + 1], sp[:])

            with tc.tile_pool(name="odeps", bufs=1, space="PSUM") as odeps, \
                 tc.tile_pool(name="mm2ps", bufs=5, space="PSUM") as mm2ps:
                z0ps = odeps.tile([64, 8], F32, tag="vfA")
                for ct in range(4):
                    nc.tensor.matmul(z0ps[:], iwT_s[:, ct * 64:(ct + 1) * 64],
                                     s0[:, ct * 8:(ct + 1) * 8],
                                     start=(ct == 0), stop=(ct == 3))
                z_sb = ode.tile([64, 8], F32)   # state, zT layout [h, b]
                nc.scalar.activation(z_sb[:], z0ps[:], AF.Identity, bias=ib_s[:, 0:1])
                if debug_dump:
                    nc.gpsimd.dma_start(dbg_pooled[:], pooled[:])
                    nc.gpsimd.dma_start(dbg_p2T[:], p2T[:])
                    nc.gpsimd.dma_start(dbg_s0[:], s0[:])
                    nc.gpsimd.dma_start(dbg_dx[:], dxtab[:])
                    nc.gpsimd.dma_start(dbg_z0[:], z_sb[:])
                    kdmp = ode.tile([64, 32], F32)
                    u2dmp = ode.tile([128, 512], BF16)
                    udmp = ode.tile([128, 8], BF16)

                import contextlib

                ustep = 2 if (not unroll and nsteps % 2 == 0) else 1

                def loop_iter():
                    if unroll:
                        for n in range(nsteps):
                            yield n, 0
                    else:
                        with tc.For_i(0, nsteps // ustep) as it:
                            for j in range(ustep):
                                yield it, j

                zero1 = ode.tile([128, 1], F32)
                nc.gpsimd.memset(zero1[:], 0.0)
                z_bf = ode.tile([64, 8], BF16)
                nc.vector.tensor_copy(z_bf[:], z_sb[:])

                for it, j in loop_iter():
                    dxs = stg.tile([128, 96], BF16, tag="dxs")
                    if unroll:
                        nc.vector.tensor_copy(dxs[:], dxtab[:, it * 64:it * 64 + 96])
                    else:
                        idx = (it * 0 if timing_mode else it * (64 * ustep)) + j * 64
                        nc.vector.tensor_copy(dxs[:], dxtab[:, bass.ds(idx, 96)])
                    zcur_bf = z_bf
                    zacc = stg.tile([64, 8], F32, tag="zacc")
                    for s in range(4):
                        qoff = (0, 1, 1, 2)[s]
                        ups = odeps.tile([128, 8], F32, tag="u")
                        nc.tensor.matmul(ups[:], w1T_s[:], zcur_bf[:], start=True, stop=True)
                        ubf = stg.tile([128, 8], BF16, tag="ubf")
                        if relu_dve:
                            # relu(x + f1b) on DVE: (ups add f1b) max 0 -> bf16
                            nc.vector.scalar_tensor_tensor(
                                ubf[:], ups[:], f1b_s[:, 0:1],
                                zero1[:].broadcast_to((128, 8)),
                                op0=ALU.add, op1=ALU.max)
                        else:
                            nc.scalar.activation(ubf[:], ups[:], AF.Relu,
                                                 bias=f1b_s[:, 0:1])
                        u2s = []

                        def emit_mm2(dt):
                            # bias preloaded into PSUM, matmuls accumulate on top
                            mps = mm2ps.tile([128, 512], F32, tag="mm2")
                            nc.vector.tensor_copy(
                                mps[:].rearrange("p (h b) -> p h b", h=64),
                                b2r_s[:, dt * 64:(dt + 1) * 64].unsqueeze(-1)
                                     .broadcast_to((128, 64, 8)))
                            for h in range(64):
                                j = h * 4 + dt
                                nc.tensor.matmul(mps[:, h * 8:(h + 1) * 8],
                                                 w2sb[:, j * 128:(j + 1) * 128],
                                                 ubf[:], start=False, stop=True,
                                                 skip_group_check=True)
                            u2d = u2p.tile([128, 512], BF16, tag="u2")
                            if abl_no_act:
                                nc.vector.tensor_copy(u2d[:], mps[:])
                            else:
                                nc.scalar.activation(u2d[:], mps[:], AF.Tanh)
                            u2s.append(u2d[:].rearrange("p (h b) -> p h b", h=64))

                        def emit_ctr(vt, dts, start):
                            for b in range(abl_nb):
                                for i, dt in enumerate(dts):
                                    dte = min(dt, abl_ndt - 1)
                                    rhs = dxs[:, qoff * 32 + dt * 8 + b:
                                              qoff * 32 + dt * 8 + b + 1]
                                    nc.tensor.matmul(vt[:, b:b + 1], u2s[dte][:, :, b],
                                                     rhs, start=(start and i == 0),
                                                     stop=(i == len(dts) - 1),
                                                     skip_group_check=True)

                        vfA = odeps.tile([64, 8], F32, tag="vfA")
                        vfB = odeps.tile([64, 8], F32, tag="vfB")
                        if split_ctr:
                            for dt in range(min(3, abl_ndt)):
                                emit_mm2(dt)
                            emit_ctr(vfA, (0, 1, 2), True)
                            if abl_ndt == 4:
                                emit_mm2(3)
                            emit_ctr(vfB, (3,), True)
                        else:
                            for dt in range(abl_ndt):
                                emit_mm2(dt)
                            emit_ctr(vfA, (0, 1), True)
                            emit_ctr(vfB, (2, 3), True)
                        if debug_dump:
                            nc.scalar.copy(kdmp[:, s * 8:(s + 1) * 8], vfA[:])
                            nc.vector.tensor_tensor(kdmp[:, s * 8:(s + 1) * 8],
                                                    kdmp[:, s * 8:(s + 1) * 8], vfB[:],
                                                    op=ALU.add)
                        ws = DT / 6.0 * (1.0, 2.0, 2.0, 1.0)[s]
                        base = z_sb if s == 0 else zacc
                        if s < 3:
                            nc.vector.scalar_tensor_tensor(zacc[:], vfA[:], ws, base[:],
                                                           op0=ALU.mult, op1=ALU.add)
                            nc.vector.scalar_tensor_tensor(zacc[:], vfB[:], ws, zacc[:],
                                                           op0=ALU.mult, op1=ALU.add)
                            cs = (DT / 2, DT / 2, DT)[s]
                            zargf = stg.tile([64, 8], F32, tag="zargf")
                            nc.vector.scalar_tensor_tensor(zargf[:], vfA[:], cs, z_sb[:],
                                                           op0=ALU.mult, op1=ALU.add)
                            zarg = stg.tile([64, 8], BF16, tag="zarg")
                            nc.vector.scalar_tensor_tensor(zarg[:], vfB[:], cs, zargf[:],
                                                           op0=ALU.mult, op1=ALU.add)
                            zcur_bf = zarg
                        else:
                            # fused tail: z_new = zacc + ws*vfA + ws*vfB, written
                            # straight into the loop-carried tiles (bf16 first:
                            # it gates the next step's mm1)
                            ztmp = stg.tile([64, 8], F32, tag="ztmp")
                            nc.vector.scalar_tensor_tensor(ztmp[:], vfA[:], ws, zacc[:],
                                                           op0=ALU.mult, op1=ALU.add)
                            nc.vector.scalar_tensor_tensor(z_bf[:], vfB[:], ws, ztmp[:],
                                                           op0=ALU.mult, op1=ALU.add)
                            nc.vector.scalar_tensor_tensor(z_sb[:], vfB[:], ws, ztmp[:],
                                                           op0=ALU.mult, op1=ALU.add)

                # ---------------- output head ----------------
                if debug_dump:
                    nc.gpsimd.dma_start(dbg_zf[:], z_sb[:])
                    nc.gpsimd.dma_start(dbg_k[:], kdmp[:])
                    nc.gpsimd.dma_start(dbg_u2[:], u2dmp[:])
                    nc.gpsimd.dma_start(dbg_u[:], udmp[:])
                ops_ = odeps.tile([2, 8], F32, tag="u")
                nc.tensor.matmul(ops_[:], owT_s[:, :], z_sb[:], start=True, stop=True)
                osb = ode.tile([2, 8], F32)
                nc.scalar.activation(osb[:], ops_[:], AF.Identity, bias=ob_s[:, 0:1])
                dst = _ap(out_d[:], 0, [(1, 2), (2, 8)])
                nc.gpsimd.dma_start(dst, osb[:])

        cpool.__exit__(None, None, None)

    nc.compile()
    return nc


_CACHE = {}


class _Runner:
    """Persistent PJRT executor: jit+shard_map built once, weights resident
    on device across calls (only x + tiny donated output buffers move)."""

    def __init__(self):
        import jax
        from jax.sharding import Mesh, PartitionSpec, NamedSharding
        from jax.experimental.shard_map import shard_map
        from concourse import bass2jax as b2j

        b2j.install_neuronx_cc_hook()
        nc = _build()
        self.nc = nc
        self.dbg_name = None
        if nc.dbg_addr is not None:
            if nc.dbg_callbacks:
                raise RuntimeError("dbg_callbacks unsupported in cached runner")
            self.dbg_name = nc.dbg_addr.name
        partition_name = (nc.partition_id_tensor.name
                          if nc.partition_id_tensor else None)
        in_names, out_names, out_avals, zero_shapes = [], [], [], []
        for alloc in nc.m.functions[0].allocations:
            if not isinstance(alloc, mybir.MemoryLocationSet):
                continue
            name = alloc.memorylocations[0].name
            if alloc.kind == "ExternalInput":
                if name != partition_name:
                    in_names.append(name)
            elif alloc.kind == "ExternalOutput":
                shape = tuple(alloc.tensor_shape)
                dtype = mybir.dt.np(alloc.dtype)
                out_names.append(name)
                out_avals.append(jax.core.ShapedArray(shape, dtype))
                zero_shapes.append((shape, dtype))
        self.param_names = list(in_names)
        self.out_names = out_names
        self.zero_shapes = zero_shapes
        n_params = len(in_names)
        n_outs = len(out_names)
        all_in_names = in_names + out_names
        if partition_name is not None:
            all_in_names.append(partition_name)

        def _body(*args):
            operands = list(args)
            if partition_name is not None:
                operands.append(b2j.partition_id_tensor())
            outs = b2j._bass_exec_p.bind(
                *operands,
                out_avals=tuple(out_avals),
                in_names=tuple(all_in_names),
                out_names=tuple(out_names),
                lowering_input_output_aliases=(),
                sim_require_finite=True,
                sim_require_nnan=True,
                nc=nc,
            )
            return tuple(outs)

        devices = jax.devices()[:N_CORES]
        assert len(devices) == N_CORES
        self.mesh = Mesh(np.asarray(devices), ("core",))
        self.sharding = NamedSharding(self.mesh, PartitionSpec("core"))
        in_specs = (PartitionSpec("core"),) * (n_params + n_outs)
        out_specs = (PartitionSpec("core"),) * n_outs
        self.sharded = jax.jit(
            shard_map(_body, mesh=self.mesh, in_specs=in_specs,
                      out_specs=out_specs, check_rep=False),
            donate_argnums=tuple(range(n_params, n_params + n_outs)),
            keep_unused=True,
        )
        self.wkey = None
        self.static_dev = None
        self._device_put = jax.device_put

    def prep_weights(self, inputs):
        key = tuple(id(inputs[k]) for k in sorted(inputs) if k != "x")
        if key == self.wkey:
            return
        sh = _shared_inputs(inputs)
        if self.dbg_name is not None:
            sh[self.dbg_name] = np.zeros((1, 2), np.uint32)
        dev = {}
        for name in self.param_names:
            if name == "x_pad":
                continue
            a = sh[name]
            g = np.broadcast_to(a[None], (N_CORES,) + a.shape).reshape(
                (N_CORES * a.shape[0],) + a.shape[1:])
            dev[name] = self._device_put(np.ascontiguousarray(g), self.sharding)
        for v in dev.values():
            v.block_until_ready()
        self.static_dev = dev
        self.wkey = key

    def __call__(self, inputs):
        self.prep_weights(inputs)
        xg = _x_global(inputs["x"])
        args = [xg if n == "x_pad" else self.static_dev[n]
                for n in self.param_names]
        # the kernel writes every element of its outputs, so the donated
        # "zero" buffers never need to actually be zero: recycle last call's
        # output arrays to skip the host->device transfer.
        zouts = getattr(self, "_prev_outs", None)
        if zouts is None:
            zouts = [np.zeros((N_CORES * s[0],) + tuple(s[1:]), d)
                     for (s, d) in self.zero_shapes]
        outs = self.sharded(*args, *zouts)
        oi = self.out_names.index("out")
        res = np.asarray(outs[oi])  # [64, 2]
        self._prev_outs = list(outs)
        return res


def _shared_inputs(inputs):
    bf = ml_dtypes.bfloat16
    c1w = np.asarray(inputs["conv1_w"], np.float32)
    c2w = np.asarray(inputs["conv2_w"], np.float32)
    sh = {
        "w1col": np.ascontiguousarray(c1w.reshape(32, 25).T.astype(bf)),
        "c1b": np.asarray(inputs["conv1_b"], np.float32).reshape(32, 1),
        "w2taps": np.ascontiguousarray(
            np.concatenate([c2w[:, :, dy, dx].T for dy in range(3) for dx in range(3)],
                           axis=1).astype(bf)),
        "c2b": np.asarray(inputs["conv2_b"], np.float32).reshape(32, 1),
        "a1w": np.ascontiguousarray(
            (np.asarray(inputs["att_fc1_w"], np.float32) / 1024.0).T),
        "a1b": np.asarray(inputs["att_fc1_b"], np.float32).reshape(4, 1),
        "a2w": np.ascontiguousarray(np.asarray(inputs["att_fc2_w"], np.float32).T),
        "a2b": np.asarray(inputs["att_fc2_b"], np.float32).reshape(32, 1),
        "HT": np.ascontiguousarray(_make_H().T),
        "iwT": np.ascontiguousarray(
            np.asarray(inputs["initial_w"], np.float32).T.reshape(4, 128, 64)
              .transpose(1, 0, 2).reshape(128, 256)),
        "ib": np.asarray(inputs["initial_b"], np.float32).reshape(64, 1),
        "w1T": np.ascontiguousarray(np.asarray(inputs["f1_w"], np.float32).T.astype(bf)),
        "f1b": np.asarray(inputs["f1_b"], np.float32).reshape(128, 1),
        "w2T": np.ascontiguousarray(np.asarray(inputs["f2_w"], np.float32).T.astype(bf)),
        "b2r": np.ascontiguousarray(
            np.asarray(inputs["f2_b"], np.float32).reshape(64, 4, 128)
              .transpose(2, 1, 0).reshape(128, 256)),
        "owT": np.ascontiguousarray(np.asarray(inputs["out_w"], np.float32).T),
        "ob": np.asarray(inputs["out_b"], np.float32).reshape(2, 1),
        "idm": np.eye(32, dtype=np.float32),
    }
    return sh


def _x_shard(x, core):
    bf = ml_dtypes.bfloat16
    xs = np.asarray(x, np.float32)[core * BPC:(core + 1) * BPC, 0]  # [8,32,128]
    xp = np.zeros((36, 8, 132), np.float32)
    xp[2:34, :, 2:130] = xs.transpose(1, 0, 2)
    return np.ascontiguousarray(xp.reshape(36, 8 * 132).astype(bf))


def _x_global(x):
    """All 8 core shards stacked on axis 0: [8*36, 8*132] bf16."""
    bf = ml_dtypes.bfloat16
    xs = np.asarray(x, np.float32)[:, 0].reshape(N_CORES, BPC, 32, 128)
    xp = np.zeros((N_CORES, 36, BPC, 132), np.float32)
    xp[:, 2:34, :, 2:130] = xs.transpose(0, 2, 1, 3)
    return xp.reshape(N_CORES * 36, BPC * 132).astype(bf)


def kernel(**inputs):
    if "runner" not in _CACHE:
        _CACHE["runner"] = _Runner()
    return _CACHE["runner"](inputs)


if __name__ == "__main__":
    rng = np.random.default_rng(0)
    ins = {
        "x": rng.standard_normal((64, 1, 32, 128)).astype(np.float32),
        "conv1_w": (rng.standard_normal((32, 1, 5, 5)) * 0.05).astype(np.float32),
        "conv1_b": np.zeros(32, np.float32),
        "conv2_w": (rng.standard_normal((32, 32, 3, 3)) * 0.05).astype(np.float32),
        "conv2_b": np.zeros(32, np.float32),
        "att_fc1_w": (rng.standard_normal((4, 32)) * 0.05).astype(np.float32),
        "att_fc1_b": np.zeros(4, np.float32),
        "att_fc2_w": (rng.standard_normal((32, 4)) * 0.05).astype(np.float32),
        "att_fc2_b": np.zeros(32, np.float32),
        "initial_w": (rng.standard_normal((64, 512)) * 0.05).astype(np.float32),
        "initial_b": np.zeros(64, np.float32),
        "f1_w": (rng.standard_normal((128, 64)) * 0.05).astype(np.float32),
        "f1_b": np.zeros(128, np.float32),
        "f2_w": (rng.standard_normal((512 * 64, 128)) * 0.05).astype(np.float32),
        "f2_b": np.zeros(512 * 64, np.float32),
        "out_w": (rng.standard_normal((2, 64)) * 0.05).astype(np.float32),
        "out_b": np.zeros(2, np.float32),
    }
    out = kernel(**ins)
    print("kernel output", out.shape, out[:2])



# revision 9
# speedup vs baseline: 1.5217x; 1.5217x over previous
"""Trainium2 Bass kernel for nn_CNN_NCDE_Model (CNN -> channel attention ->
natural-cubic-spline NCDE integrated with fixed-step RK4).

Strategy: pure data parallelism over batch (64 -> 8 cores x 8 images).
The spline coefficient solve + derivative evaluation collapses into one
constant matrix H[253,64] applied to seq (host-precomputed from the
tridiagonal system; data-independent), so the whole pre-ODE stage is a
small set of matmuls. The ODE scan (126 RK4 steps = 504 vector-field
evals) is the dominant cost: each eval is [8,64]@f1 -> relu ->
[8,128]@f2(32768x128, bf16) -> +bias -> tanh -> contraction with dX.
f2 weights stay resident in SBUF (bf16) and are streamed through the PE
as 256 stationary tiles per eval; the dX contraction runs on the PE as
per-batch M=1 accumulated matmuls, fully avoiding cross-partition
reductions.
"""
import numpy as np
import ml_dtypes

import concourse.bacc as bacc
import concourse.bass as bass
import concourse.mybir as mybir
import concourse.tile as tile
from concourse.bass_utils import run_bass_kernel_spmd

F32 = mybir.dt.float32
BF16 = mybir.dt.bfloat16
AF = mybir.ActivationFunctionType
ALU = mybir.AluOpType

N_CORES = 8
BPC = 8            # batch per core
L = 64             # sequence length after pooling
NQ = 253           # quarter-time points t=q/4, q=0..252
NSTEPS = 126
DT = 0.5


def _make_H():
    """H[q,l] with dX(t_q)[b,c] = sum_l H[q,l]*seq[b,l,c] (natural cubic)."""
    n = L - 2
    A = 4.0 * np.eye(n) + np.eye(n, k=1) + np.eye(n, k=-1)
    Ainv = np.linalg.inv(A)
    R = np.zeros((n, L))
    for j in range(n):
        R[j, j] += 6.0
        R[j, j + 1] += -12.0
        R[j, j + 2] += 6.0
    Mmat = np.zeros((L, L))
    Mmat[1:L - 1, :] = Ainv @ R
    H = np.zeros((NQ, L))
    for q in range(NQ):
        seg = min(q // 4, L - 2)
        fr = q / 4.0 - seg
        al = -1.0 / 3.0 + fr - fr * fr / 2.0
        be = -1.0 / 6.0 + fr * fr / 2.0
        H[q, seg] += -1.0
        H[q, seg + 1] += 1.0
        H[q, :] += al * Mmat[seg, :] + be * Mmat[seg + 1, :]
    return H.astype(np.float32)


def _ap(t_ap, offset, dims):
    return bass.AP(t_ap.tensor, offset, [list(d) for d in dims])


def _build(phase=99, nsteps=NSTEPS, debug_dump=False, unroll=False, timing_mode=False, relu_dve=True, split_ctr=True, abl_ndt=4, abl_nb=BPC, abl_no_act=False):
    nc = bacc.Bacc("TRN2", target_bir_lowering=False, debug=True)

    def din(name, shape, dt):
        return nc.dram_tensor(name, shape, dt, kind="ExternalInput")

    x_pad = din("x_pad", [36, 8 * 132], BF16)      # padded input, h x (img,w)
    w1col = din("w1col", [25, 32], BF16)           # conv1 as K=25 lhsT
    c1b = din("c1b", [32, 1], F32)
    w2taps = din("w2taps", [32, 9 * 32], BF16)     # conv2 per-tap lhsT
    c2b = din("c2b", [32, 1], F32)
    a1w = din("a1w", [32, 4], F32)                 # att fc1 lhsT (pre-scaled /1024)
    a1b = din("a1b", [4, 1], F32)
    a2w = din("a2w", [4, 32], F32)
    a2b = din("a2b", [32, 1], F32)
    HTd = din("HT", [64, NQ], F32)                 # H^T
    iwT = din("iwT", [128, 4 * 64], F32)           # initial_w^T tiles
    ibd = din("ib", [64, 1], F32)
    w1T = din("w1T", [64, 128], BF16)              # f1_w^T
    f1bd = din("f1b", [128, 1], F32)
    w2T = din("w2T", [128, 32768], BF16)           # f2_w^T
    b2r = din("b2r", [128, 4 * 64], F32)           # f2_b as [c, (dt,h)]
    owT = din("owT", [64, 2], F32)
    obd = din("ob", [2, 1], F32)
    idmd = din("idm", [32, 32], F32)
    out_d = nc.dram_tensor("out", [BPC, 2], F32, kind="ExternalOutput")
    if debug_dump:
        dbg_pooled = nc.dram_tensor("dbg_pooled", [32, 8192], F32, kind="ExternalOutput")
        dbg_p2T = nc.dram_tensor("dbg_p2T", [64, 4096], F32, kind="ExternalOutput")
        dbg_s0 = nc.dram_tensor("dbg_s0", [128, 32], F32, kind="ExternalOutput")
        dbg_dx = nc.dram_tensor("dbg_dx", [128, NQ * 32], mybir.dt.bfloat16, kind="ExternalOutput")
        dbg_z0 = nc.dram_tensor("dbg_z0", [64, 8], F32, kind="ExternalOutput")
        dbg_k = nc.dram_tensor("dbg_k", [64, 8 * 4], F32, kind="ExternalOutput")
        dbg_u2 = nc.dram_tensor("dbg_u2", [128, 512], mybir.dt.bfloat16, kind="ExternalOutput")
        dbg_u = nc.dram_tensor("dbg_u", [128, 8], mybir.dt.bfloat16, kind="ExternalOutput")
        dbg_zf = nc.dram_tensor("dbg_zf", [64, 8], F32, kind="ExternalOutput")

    with tile.TileContext(nc) as tc:
        cpool = tc.tile_pool(name="consts", bufs=1)
        cp = cpool.__enter__()

        def load_const(dram, shape, dt):
            t = cp.tile(shape, dt, tag=f"c_{dram.name}")
            nc.gpsimd.dma_start(t[:], dram[:])
            return t

        w1col_s = load_const(w1col, [25, 32], BF16)
        c1b_s = load_const(c1b, [32, 1], F32)
        w2taps_s = load_const(w2taps, [32, 288], BF16)
        c2b_s = load_const(c2b, [32, 1], F32)
        a1w_s = load_const(a1w, [32, 4], F32)
        a1b_s = load_const(a1b, [4, 1], F32)
        a2w_s = load_const(a2w, [4, 32], F32)
        a2b_s = load_const(a2b, [32, 1], F32)
        HT_s = load_const(HTd, [64, NQ], F32)
        iwT_s = load_const(iwT, [128, 256], F32)
        ib_s = load_const(ibd, [64, 1], F32)
        w1T_s = load_const(w1T, [64, 128], BF16)
        f1b_s = load_const(f1bd, [128, 1], F32)
        b2r_s = load_const(b2r, [128, 256], F32)
        owT_s = load_const(owT, [64, 2], F32)
        ob_s = load_const(obd, [2, 1], F32)
        idm_s = load_const(idmd, [32, 32], F32)
        pooled = cp.tile([32, 8192], F32)
        pooled_r = pooled[:].rearrange("p (i hp w) -> p i hp w", i=8, hp=16, w=64)

        # ---------------- CNN ----------------
        if phase >= 1:
          with tc.tile_pool(name="cnn", bufs=1) as cnn, \
             tc.tile_pool(name="cnn2", bufs=2) as cnn2, \
             tc.tile_pool(name="cnnps", bufs=2, space="PSUM") as cnnps:
            c1pad = cnn.tile([32, 8 * 34 * 130], BF16)
            nc.gpsimd.memset(c1pad[:], 0.0)
            c1pad_r = c1pad[:].rearrange("p (i h w) -> p i h w", i=8, h=34, w=130)

            # conv1, processed in 4 chunks of 8 output rows
            for hc in range(4):
                h0 = hc * 8
                imcol = cnn2.tile([25, 8192], BF16, tag="imcol")
                for dy in range(5):
                    src = _ap(x_pad[:], (h0 + dy) * 1056,
                              [(1, 5), (1056, 8), (132, 8), (1, 128)])
                    nc.gpsimd.dma_start(imcol[dy * 5:(dy + 1) * 5, :], src)
                for c in range(16):
                    h = h0 + c // 2
                    ihalf = c % 2
                    ps = cnnps.tile([32, 512], F32, tag="c1")
                    nc.tensor.matmul(ps[:], w1col_s[:], imcol[:, c * 512:(c + 1) * 512],
                                     start=True, stop=True)
                    dest = c1pad_r[:, 4 * ihalf:4 * ihalf + 4, 1 + h, 1:129]
                    nc.scalar.activation(dest, ps[:].rearrange("p (i w) -> p i w", i=4),
                                         AF.Relu, bias=c1b_s[:, 0:1])

            # conv2 (tap-accumulated) + relu + maxpool, per image / 4-row chunk
            for img in range(8):
                for hc in range(8):
                    h0 = hc * 4
                    ps2 = cnnps.tile([32, 512], F32, tag="c2")
                    for tap in range(9):
                        dy, dx = tap // 3, tap % 3
                        rhs = c1pad_r[:, img, h0 + dy:h0 + dy + 4, dx:dx + 128]
                        nc.tensor.matmul(ps2[:], w2taps_s[:, tap * 32:(tap + 1) * 32],
                                         rhs, start=(tap == 0), stop=(tap == 8))
                    c2c = cnn2.tile([32, 512], F32, tag="c2out")
                    nc.scalar.activation(c2c[:], ps2[:], AF.Relu, bias=c2b_s[:, 0:1])
                    c2r = c2c[:].rearrange("p (h a w b) -> p h a w b", h=2, a=2, w=64, b=2)
                    t1 = cnn2.tile([32, 128], F32, tag="pa")
                    t1r = t1[:].rearrange("p (h w) -> p h w", h=2)
                    t2 = cnn2.tile([32, 128], F32, tag="pb")
                    t2r = t2[:].rearrange("p (h w) -> p h w", h=2)
                    nc.vector.tensor_tensor(t1r, c2r[:, :, 0, :, 0], c2r[:, :, 0, :, 1], op=ALU.max)
                    nc.vector.tensor_tensor(t2r, c2r[:, :, 1, :, 0], c2r[:, :, 1, :, 1], op=ALU.max)
                    dest = pooled_r[:, img, h0 // 2:h0 // 2 + 2, :]
                    nc.vector.tensor_tensor(dest, t1r, t2r, op=ALU.max)

        # ---------------- attention ----------------
        if phase >= 2:
          with tc.tile_pool(name="att", bufs=1) as att, \
             tc.tile_pool(name="attps", bufs=1, space="PSUM") as attps:
            satt = att.tile([32, 8], F32)
            nc.vector.tensor_reduce(satt[:], pooled[:].rearrange("p (i f) -> p i f", i=8),
                                    axis=mybir.AxisListType.X, op=ALU.add)
            a1ps = attps.tile([4, 8], F32, tag="a1")
            nc.tensor.matmul(a1ps[:], a1w_s[:], satt[:], start=True, stop=True)
            att1 = att.tile([4, 8], F32)
            nc.scalar.activation(att1[:], a1ps[:], AF.Relu, bias=a1b_s[:, 0:1])
            a2ps = attps.tile([32, 8], F32, tag="a2")
            nc.tensor.matmul(a2ps[:], a2w_s[:], att1[:], start=True, stop=True)
            attw = att.tile([32, 8], F32)
            nc.scalar.activation(attw[:], a2ps[:], AF.Sigmoid, bias=a2b_s[:, 0:1])
            nc.vector.tensor_tensor(
                pooled[:].rearrange("p (i f) -> p i f", i=8),
                pooled[:].rearrange("p (i f) -> p i f", i=8),
                attw[:].unsqueeze(-1).broadcast_to((32, 8, 1024)),
                op=ALU.mult)

        # ---------------- spline/dX table + z0 + ODE ----------------
        if phase >= 3:
          with tc.tile_pool(name="ode", bufs=1) as ode, \
             tc.tile_pool(name="seqp", bufs=2) as seqp, \
             tc.tile_pool(name="stg", bufs=2) as stg, \
             tc.tile_pool(name="u2p", bufs=5) as u2p:

            w2sb = ode.tile([128, 32768], BF16)
            for ch in range(8):
                nc.gpsimd.dma_start(w2sb[:, ch * 4096:(ch + 1) * 4096],
                                    w2T[:, ch * 4096:(ch + 1) * 4096])
            dxtab = ode.tile([128, NQ * 32], BF16)   # [c, (q, dt, b)]
            dxtab_r = dxtab[:].rearrange("p (q c b) -> p q c b", q=NQ, c=4, b=8)

            p2T = ode.tile([64, 8 * 512], F32)   # seq, [w][img][oc*16+hp]
            p2T_r = p2T[:].rearrange("w (i o h) -> w i o h", i=8, o=32, h=16)
            with tc.tile_pool(name="dxps", bufs=2, space="PSUM") as dxps:
                for img in range(8):
                    for hp in range(16):
                        tp = dxps.tile([64, 32], F32, tag="tp")
                        nc.tensor.transpose(tp[:], pooled_r[:, img, hp, :], idm_s[:, :])
                        nc.scalar.copy(p2T_r[:, img, :, hp], tp[:])
                for b in range(BPC):
                    for ct in range(4):
                        dps = dxps.tile([128, NQ], F32, tag="dx")
                        nc.tensor.matmul(dps[:], p2T[:, b * 512 + ct * 128:b * 512 + (ct + 1) * 128],
                                         HT_s[:], start=True, stop=True)
                        nc.scalar.copy(dxtab_r[:, :, ct, b], dps[:])
                s0 = ode.tile([128, 32], F32)
                for b in range(BPC):
                    for ct in range(4):
                        sp = dxps.tile([128, 1], F32, tag="s0p")
                        nc.tensor.transpose(
                            sp[:], p2T[0:1, b * 512 + ct * 128:b * 512 + (ct + 1) * 128],
                            idm_s[0:1, 0:1])
                        nc.scalar.copy(s0[:, ct * 8 + b:ct * 8 + b + 1], sp[:])

            with tc.tile_pool(name="odeps", bufs=1, space="PSUM") as odeps, \
                 tc.tile_pool(name="mm2ps", bufs=5, space="PSUM") as mm2ps:
                z0ps = odeps.tile([64, 8], F32, tag="vfA")
                for ct in range(4):
                    nc.tensor.matmul(z0ps[:], iwT_s[:, ct * 64:(ct + 1) * 64],
                                     s0[:, ct * 8:(ct + 1) * 8],
                                     start=(ct == 0), stop=(ct == 3))
                z_sb = ode.tile([64, 8], F32)   # state, zT layout [h, b]
                nc.scalar.activation(z_sb[:], z0ps[:], AF.Identity, bias=ib_s[:, 0:1])
                if debug_dump:
                    nc.gpsimd.dma_start(dbg_pooled[:], pooled[:])
                    nc.gpsimd.dma_start(dbg_p2T[:], p2T[:])
                    nc.gpsimd.dma_start(dbg_s0[:], s0[:])
                    nc.gpsimd.dma_start(dbg_dx[:], dxtab[:])
                    nc.gpsimd.dma_start(dbg_z0[:], z_sb[:])
                    kdmp = ode.tile([64, 32], F32)
                    u2dmp = ode.tile([128, 512], BF16)
                    udmp = ode.tile([128, 8], BF16)

                import contextlib

                ustep = 2 if (not unroll and nsteps % 2 == 0) else 1

                def loop_iter():
                    if unroll:
                        for n in range(nsteps):
                            yield n, 0
                    else:
                        with tc.For_i(0, nsteps // ustep) as it:
                            for j in range(ustep):
                                yield it, j

                zero1 = ode.tile([128, 1], F32)
                nc.gpsimd.memset(zero1[:], 0.0)
                z_bf = ode.tile([64, 8], BF16)
                nc.vector.tensor_copy(z_bf[:], z_sb[:])

                for it, j in loop_iter():
                    dxs = stg.tile([128, 96], BF16, tag="dxs")
                    if unroll:
                        nc.vector.tensor_copy(dxs[:], dxtab[:, it * 64:it * 64 + 96])
                    else:
                        idx = (it * 0 if timing_mode else it * (64 * ustep)) + j * 64
                        nc.vector.tensor_copy(dxs[:], dxtab[:, bass.ds(idx, 96)])
                    zcur_bf = z_bf
                    zacc = stg.tile([64, 8], F32, tag="zacc")
                    for s in range(4):
                        qoff = (0, 1, 1, 2)[s]
                        ups = odeps.tile([128, 8], F32, tag="u")
                        nc.tensor.matmul(ups[:], w1T_s[:], zcur_bf[:], start=True, stop=True)
                        ubf = stg.tile([128, 8], BF16, tag="ubf")
                        if relu_dve:
                            # relu(x + f1b) on DVE: (ups add f1b) max 0 -> bf16
                            nc.vector.scalar_tensor_tensor(
                                ubf[:], ups[:], f1b_s[:, 0:1],
                                zero1[:].broadcast_to((128, 8)),
                                op0=ALU.add, op1=ALU.max)
                        else:
                            nc.scalar.activation(ubf[:], ups[:], AF.Relu,
                                                 bias=f1b_s[:, 0:1])
                        u2s = []

                        def emit_mm2(dt):
                            # bias preloaded into PSUM, matmuls accumulate on top
                            mps = mm2ps.tile([128, 512], F32, tag="mm2")
                            nc.vector.tensor_copy(
                                mps[:].rearrange("p (h b) -> p h b", h=64),
                                b2r_s[:, dt * 64:(dt + 1) * 64].unsqueeze(-1)
                                     .broadcast_to((128, 64, 8)))
                            for h in range(64):
                                j = h * 4 + dt
                                nc.tensor.matmul(mps[:, h * 8:(h + 1) * 8],
                                                 w2sb[:, j * 128:(j + 1) * 128],
                                                 ubf[:], start=False, stop=True,
                                                 skip_group_check=True)
                            u2d = u2p.tile([128, 512], BF16, tag="u2")
                            if abl_no_act:
                                nc.vector.tensor_copy(u2d[:], mps[:])
                            else:
                                nc.scalar.activation(u2d[:], mps[:], AF.Tanh)
                            u2s.append(u2d[:].rearrange("p (h b) -> p h b", h=64))

                        def emit_ctr(vt, dts, start):
                            for b in range(abl_nb):
                                for i, dt in enumerate(dts):
                                    dte = min(dt, abl_ndt - 1)
                                    rhs = dxs[:, qoff * 32 + dt * 8 + b:
                                              qoff * 32 + dt * 8 + b + 1]
                                    nc.tensor.matmul(vt[:, b:b + 1], u2s[dte][:, :, b],
                                                     rhs, start=(start and i == 0),
                                                     stop=(i == len(dts) - 1),
                                                     skip_group_check=True)

                        vfA = odeps.tile([64, 8], F32, tag="vfA")
                        vfB = odeps.tile([64, 8], F32, tag="vfB")
                        if split_ctr:
                            for dt in range(min(3, abl_ndt)):
                                emit_mm2(dt)
                            emit_ctr(vfA, (0, 1, 2), True)
                            if abl_ndt == 4:
                                emit_mm2(3)
                            emit_ctr(vfB, (3,), True)
                        else:
                            for dt in range(abl_ndt):
                                emit_mm2(dt)
                            emit_ctr(vfA, (0, 1), True)
                            emit_ctr(vfB, (2, 3), True)
                        if debug_dump:
                            nc.scalar.copy(kdmp[:, s * 8:(s + 1) * 8], vfA[:])
                            nc.vector.tensor_tensor(kdmp[:, s * 8:(s + 1) * 8],
                                                    kdmp[:, s * 8:(s + 1) * 8], vfB[:],
                                                    op=ALU.add)
                        ws = DT / 6.0 * (1.0, 2.0, 2.0, 1.0)[s]
                        base = z_sb if s == 0 else zacc
                        if s < 3:
                            nc.vector.scalar_tensor_tensor(zacc[:], vfA[:], ws, base[:],
                                                           op0=ALU.mult, op1=ALU.add)
                            nc.vector.scalar_tensor_tensor(zacc[:], vfB[:], ws, zacc[:],
                                                           op0=ALU.mult, op1=ALU.add)
                            cs = (DT / 2, DT / 2, DT)[s]
                            zargf = stg.tile([64, 8], F32, tag="zargf")
                            nc.vector.scalar_tensor_tensor(zargf[:], vfA[:], cs, z_sb[:],
                                                           op0=ALU.mult, op1=ALU.add)
                            zarg = stg.tile([64, 8], BF16, tag="zarg")
                            nc.vector.scalar_tensor_tensor(zarg[:], vfB[:], cs, zargf[:],
                                                           op0=ALU.mult, op1=ALU.add)
                            zcur_bf = zarg
                        else:
                            # fused tail: z_new = zacc + ws*vfA + ws*vfB, written
                            # straight into the loop-carried tiles (bf16 first:
                            # it gates the next step's mm1)
                            ztmp = stg.tile([64, 8], F32, tag="ztmp")
                            nc.vector.scalar_tensor_tensor(ztmp[:], vfA[:], ws, zacc[:],
                                                           op0=ALU.mult, op1=ALU.add)
                            nc.vector.scalar_tensor_tensor(z_bf[:], vfB[:], ws, ztmp[:],
                                                           op0=ALU.mult, op1=ALU.add)
                            nc.vector.scalar_tensor_tensor(z_sb[:], vfB[:], ws, ztmp[:],
                                                           op0=ALU.mult, op1=ALU.add)

                # ---------------- output head ----------------
                if debug_dump:
                    nc.gpsimd.dma_start(dbg_zf[:], z_sb[:])
                    nc.gpsimd.dma_start(dbg_k[:], kdmp[:])
                    nc.gpsimd.dma_start(dbg_u2[:], u2dmp[:])
                    nc.gpsimd.dma_start(dbg_u[:], udmp[:])
                ops_ = odeps.tile([2, 8], F32, tag="u")
                nc.tensor.matmul(ops_[:], owT_s[:, :], z_sb[:], start=True, stop=True)
                osb = ode.tile([2, 8], F32)
                nc.scalar.activation(osb[:], ops_[:], AF.Identity, bias=ob_s[:, 0:1])
                dst = _ap(out_d[:], 0, [(1, 2), (2, 8)])
                nc.gpsimd.dma_start(dst, osb[:])

        cpool.__exit__(None, None, None)

    nc.compile()
    return nc


_CACHE = {}


class _Runner:
    """Persistent PJRT executor: jit+shard_map built once, weights resident
    on device across calls (only x + tiny donated output buffers move)."""

    def __init__(self):
        import jax
        from jax.sharding import Mesh, PartitionSpec, NamedSharding
        from jax.experimental.shard_map import shard_map
        from concourse import bass2jax as b2j

        b2j.install_neuronx_cc_hook()
        nc = _build()
        self.nc = nc
        self.dbg_name = None
        if nc.dbg_addr is not None:
            if nc.dbg_callbacks:
                raise RuntimeError("dbg_callbacks unsupported in cached runner")
            self.dbg_name = nc.dbg_addr.name
        partition_name = (nc.partition_id_tensor.name
                          if nc.partition_id_tensor else None)
        in_names, out_names, out_avals, zero_shapes = [], [], [], []
        for alloc in nc.m.functions[0].allocations:
            if not isinstance(alloc, mybir.MemoryLocationSet):
                continue
            name = alloc.memorylocations[0].name
            if alloc.kind == "ExternalInput":
                if name != partition_name:
                    in_names.append(name)
            elif alloc.kind == "ExternalOutput":
                shape = tuple(alloc.tensor_shape)
                dtype = mybir.dt.np(alloc.dtype)
                out_names.append(name)
                out_avals.append(jax.core.ShapedArray(shape, dtype))
                zero_shapes.append((shape, dtype))
        self.param_names = list(in_names)
        self.out_names = out_names
        self.zero_shapes = zero_shapes
        n_params = len(in_names)
        n_outs = len(out_names)
        all_in_names = in_names + out_names
        if partition_name is not None:
            all_in_names.append(partition_name)

        def _body(*args):
            operands = list(args)
            if partition_name is not None:
                operands.append(b2j.partition_id_tensor())
            outs = b2j._bass_exec_p.bind(
                *operands,
                out_avals=tuple(out_avals),
                in_names=tuple(all_in_names),
                out_names=tuple(out_names),
                lowering_input_output_aliases=(),
                sim_require_finite=True,
                sim_require_nnan=True,
                nc=nc,
            )
            return tuple(outs)

        devices = jax.devices()[:N_CORES]
        assert len(devices) == N_CORES
        self.mesh = Mesh(np.asarray(devices), ("core",))
        self.sharding = NamedSharding(self.mesh, PartitionSpec("core"))
        in_specs = (PartitionSpec("core"),) * (n_params + n_outs)
        out_specs = (PartitionSpec("core"),) * n_outs
        self.sharded = jax.jit(
            shard_map(_body, mesh=self.mesh, in_specs=in_specs,
                      out_specs=out_specs, check_rep=False),
            donate_argnums=tuple(range(n_params, n_params + n_outs)),
            keep_unused=True,
        )
        self.wkey = None
        self.static_dev = None
        self._device_put = jax.device_put

    def prep_weights(self, inputs):
        key = tuple(id(inputs[k]) for k in sorted(inputs) if k != "x")
        if key == self.wkey:
            return
        sh = _shared_inputs(inputs)
        if self.dbg_name is not None:
            sh[self.dbg_name] = np.zeros((1, 2), np.uint32)
        dev = {}
        for name in self.param_names:
            if name == "x_pad":
                continue
            a = sh[name]
            g = np.broadcast_to(a[None], (N_CORES,) + a.shape).reshape(
                (N_CORES * a.shape[0],) + a.shape[1:])
            dev[name] = self._device_put(np.ascontiguousarray(g), self.sharding)
        for v in dev.values():
            v.block_until_ready()
        self.static_dev = dev
        self.wkey = key

    def __call__(self, inputs):
        self.prep_weights(inputs)
        xg = _x_global(inputs["x"])
        args = [xg if n == "x_pad" else self.static_dev[n]
                for n in self.param_names]
        zouts = [np.zeros((N_CORES * s[0],) + tuple(s[1:]), d)
                 for (s, d) in self.zero_shapes]
        outs = self.sharded(*args, *zouts)
        oi = self.out_names.index("out")
        return np.asarray(outs[oi])  # [64, 2]


def _shared_inputs(inputs):
    bf = ml_dtypes.bfloat16
    c1w = np.asarray(inputs["conv1_w"], np.float32)
    c2w = np.asarray(inputs["conv2_w"], np.float32)
    sh = {
        "w1col": np.ascontiguousarray(c1w.reshape(32, 25).T.astype(bf)),
        "c1b": np.asarray(inputs["conv1_b"], np.float32).reshape(32, 1),
        "w2taps": np.ascontiguousarray(
            np.concatenate([c2w[:, :, dy, dx].T for dy in range(3) for dx in range(3)],
                           axis=1).astype(bf)),
        "c2b": np.asarray(inputs["conv2_b"], np.float32).reshape(32, 1),
        "a1w": np.ascontiguousarray(
            (np.asarray(inputs["att_fc1_w"], np.float32) / 1024.0).T),
        "a1b": np.asarray(inputs["att_fc1_b"], np.float32).reshape(4, 1),
        "a2w": np.ascontiguousarray(np.asarray(inputs["att_fc2_w"], np.float32).T),
        "a2b": np.asarray(inputs["att_fc2_b"], np.float32).reshape(32, 1),
        "HT": np.ascontiguousarray(_make_H().T),
        "iwT": np.ascontiguousarray(
            np.asarray(inputs["initial_w"], np.float32).T.reshape(4, 128, 64)
              .transpose(1, 0, 2).reshape(128, 256)),
        "ib": np.asarray(inputs["initial_b"], np.float32).reshape(64, 1),
        "w1T": np.ascontiguousarray(np.asarray(inputs["f1_w"], np.float32).T.astype(bf)),
        "f1b": np.asarray(inputs["f1_b"], np.float32).reshape(128, 1),
        "w2T": np.ascontiguousarray(np.asarray(inputs["f2_w"], np.float32).T.astype(bf)),
        "b2r": np.ascontiguousarray(
            np.asarray(inputs["f2_b"], np.float32).reshape(64, 4, 128)
              .transpose(2, 1, 0).reshape(128, 256)),
        "owT": np.ascontiguousarray(np.asarray(inputs["out_w"], np.float32).T),
        "ob": np.asarray(inputs["out_b"], np.float32).reshape(2, 1),
        "idm": np.eye(32, dtype=np.float32),
    }
    return sh


def _x_shard(x, core):
    bf = ml_dtypes.bfloat16
    xs = np.asarray(x, np.float32)[core * BPC:(core + 1) * BPC, 0]  # [8,32,128]
    xp = np.zeros((36, 8, 132), np.float32)
    xp[2:34, :, 2:130] = xs.transpose(1, 0, 2)
    return np.ascontiguousarray(xp.reshape(36, 8 * 132).astype(bf))


def _x_global(x):
    """All 8 core shards stacked on axis 0: [8*36, 8*132] bf16."""
    bf = ml_dtypes.bfloat16
    xs = np.asarray(x, np.float32)[:, 0].reshape(N_CORES, BPC, 32, 128)
    xp = np.zeros((N_CORES, 36, BPC, 132), np.float32)
    xp[:, 2:34, :, 2:130] = xs.transpose(0, 2, 1, 3)
    return xp.reshape(N_CORES * 36, BPC * 132).astype(bf)


def kernel(**inputs):
    if "runner" not in _CACHE:
        _CACHE["runner"] = _Runner()
    return _CACHE["runner"](inputs)


if __name__ == "__main__":
    rng = np.random.default_rng(0)
    ins = {
        "x": rng.standard_normal((64, 1, 32, 128)).astype(np.float32),
        "conv1_w": (rng.standard_normal((32, 1, 5, 5)) * 0.05).astype(np.float32),
        "conv1_b": np.zeros(32, np.float32),
        "conv2_w": (rng.standard_normal((32, 32, 3, 3)) * 0.05).astype(np.float32),
        "conv2_b": np.zeros(32, np.float32),
        "att_fc1_w": (rng.standard_normal((4, 32)) * 0.05).astype(np.float32),
        "att_fc1_b": np.zeros(4, np.float32),
        "att_fc2_w": (rng.standard_normal((32, 4)) * 0.05).astype(np.float32),
        "att_fc2_b": np.zeros(32, np.float32),
        "initial_w": (rng.standard_normal((64, 512)) * 0.05).astype(np.float32),
        "initial_b": np.zeros(64, np.float32),
        "f1_w": (rng.standard_normal((128, 64)) * 0.05).astype(np.float32),
        "f1_b": np.zeros(128, np.float32),
        "f2_w": (rng.standard_normal((512 * 64, 128)) * 0.05).astype(np.float32),
        "f2_b": np.zeros(512 * 64, np.float32),
        "out_w": (rng.standard_normal((2, 64)) * 0.05).astype(np.float32),
        "out_b": np.zeros(2, np.float32),
    }
    out = kernel(**ins)
    print("kernel output", out.shape, out[:2])



# revision 16
# speedup vs baseline: 1.6293x; 1.0707x over previous
"""Trainium2 Bass kernel for nn_CNN_NCDE_Model (CNN -> channel attention ->
natural-cubic-spline NCDE integrated with fixed-step RK4).

Strategy: pure data parallelism over batch (64 -> 8 cores x 8 images).
The spline coefficient solve + derivative evaluation collapses into one
constant matrix H[253,64] applied to seq (host-precomputed from the
tridiagonal system; data-independent), so the whole pre-ODE stage is a
small set of matmuls. The ODE scan (126 RK4 steps = 504 vector-field
evals) is the dominant cost: each eval is [8,64]@f1 -> relu ->
[8,128]@f2(32768x128, bf16) -> +bias -> tanh -> contraction with dX.
f2 weights stay resident in SBUF (bf16) and are streamed through the PE
as 256 stationary tiles per eval; the dX contraction runs on the PE as
per-batch M=1 accumulated matmuls, fully avoiding cross-partition
reductions.
"""
import numpy as np
import ml_dtypes

import concourse.bacc as bacc
import concourse.bass as bass
import concourse.mybir as mybir
import concourse.tile as tile
from concourse.bass_utils import run_bass_kernel_spmd

F32 = mybir.dt.float32
BF16 = mybir.dt.bfloat16
AF = mybir.ActivationFunctionType
ALU = mybir.AluOpType

N_CORES = 8
BPC = 8            # batch per core
L = 64             # sequence length after pooling
NSTEPS = 126
DT = 0.5

# Dormand-Prince 5(4): dt=1, knot-aligned steps (spline is smooth inside each
# unit segment), FSAL so 6 vf evals per step instead of RK4@0.5's 8 per unit.
DP5_NSTEPS = 63
DP5_C = (0.2, 0.3, 0.8, 8.0 / 9.0, 1.0)      # dX table offsets per step
DP5_NQ = 1 + DP5_NSTEPS * len(DP5_C)          # 316 groups (group 0: t=0)
DP5_A = {
    2: (1.0 / 5.0,),
    3: (3.0 / 40.0, 9.0 / 40.0),
    4: (44.0 / 45.0, -56.0 / 15.0, 32.0 / 9.0),
    5: (19372.0 / 6561.0, -25360.0 / 2187.0, 64448.0 / 6561.0, -212.0 / 729.0),
    6: (9017.0 / 3168.0, -355.0 / 33.0, 46732.0 / 5247.0, 49.0 / 176.0,
        -5103.0 / 18656.0),
}
DP5_B = (35.0 / 384.0, 0.0, 500.0 / 1113.0, 125.0 / 192.0, -2187.0 / 6784.0,
         11.0 / 84.0)
NQ = DP5_NQ


def _dp5_ts():
    return [0.0] + [n + c for n in range(DP5_NSTEPS) for c in DP5_C]


def _make_H():
    """H[q,l] with dX(t_q)[b,c] = sum_l H[q,l]*seq[b,l,c] (natural cubic),
    rows at the DP5 stage times."""
    ts = _dp5_ts()
    n = L - 2
    A = 4.0 * np.eye(n) + np.eye(n, k=1) + np.eye(n, k=-1)
    Ainv = np.linalg.inv(A)
    R = np.zeros((n, L))
    for j in range(n):
        R[j, j] += 6.0
        R[j, j + 1] += -12.0
        R[j, j + 2] += 6.0
    Mmat = np.zeros((L, L))
    Mmat[1:L - 1, :] = Ainv @ R
    H = np.zeros((len(ts), L))
    for q, t in enumerate(ts):
        seg = min(int(np.floor(t)), L - 2)
        fr = t - seg
        al = -1.0 / 3.0 + fr - fr * fr / 2.0
        be = -1.0 / 6.0 + fr * fr / 2.0
        H[q, seg] += -1.0
        H[q, seg + 1] += 1.0
        H[q, :] += al * Mmat[seg, :] + be * Mmat[seg + 1, :]
    return H.astype(np.float32)


def _ap(t_ap, offset, dims):
    return bass.AP(t_ap.tensor, offset, [list(d) for d in dims])


def _build(phase=99, nsteps=DP5_NSTEPS, debug_dump=False, unroll=False, timing_mode=False, relu_dve=True, split_ctr=True, abl_ndt=4, abl_nb=BPC, abl_no_act=False):
    nc = bacc.Bacc("TRN2", target_bir_lowering=False, debug=True)

    def din(name, shape, dt):
        return nc.dram_tensor(name, shape, dt, kind="ExternalInput")

    x_pad = din("x_pad", [36, 8 * 132], BF16)      # padded input, h x (img,w)
    w1col = din("w1col", [25, 32], BF16)           # conv1 as K=25 lhsT
    c1b = din("c1b", [32, 1], F32)
    w2taps = din("w2taps", [32, 9 * 32], BF16)     # conv2 per-tap lhsT
    c2b = din("c2b", [32, 1], F32)
    a1w = din("a1w", [32, 4], F32)                 # att fc1 lhsT (pre-scaled /1024)
    a1b = din("a1b", [4, 1], F32)
    a2w = din("a2w", [4, 32], F32)
    a2b = din("a2b", [32, 1], F32)
    HTd = din("HT", [64, NQ], F32)                 # H^T
    iwT = din("iwT", [128, 4 * 64], F32)           # initial_w^T tiles
    ibd = din("ib", [64, 1], F32)
    w1T = din("w1T", [64, 128], BF16)              # f1_w^T
    f1bd = din("f1b", [128, 1], F32)
    w2T = din("w2T", [128, 32768], BF16)           # f2_w^T
    b2r = din("b2r", [128, 4 * 64], F32)           # f2_b as [c, (dt,h)]
    owT = din("owT", [64, 2], F32)
    obd = din("ob", [2, 1], F32)
    idmd = din("idm", [32, 32], F32)
    out_d = nc.dram_tensor("out", [BPC, 2], F32, kind="ExternalOutput")
    if debug_dump:
        dbg_pooled = nc.dram_tensor("dbg_pooled", [32, 8192], F32, kind="ExternalOutput")
        dbg_p2T = nc.dram_tensor("dbg_p2T", [64, 4096], F32, kind="ExternalOutput")
        dbg_s0 = nc.dram_tensor("dbg_s0", [128, 32], F32, kind="ExternalOutput")
        dbg_dx = nc.dram_tensor("dbg_dx", [128, NQ * 32], mybir.dt.bfloat16, kind="ExternalOutput")
        dbg_z0 = nc.dram_tensor("dbg_z0", [64, 8], F32, kind="ExternalOutput")
        dbg_k = nc.dram_tensor("dbg_k", [64, 8 * 4], F32, kind="ExternalOutput")
        dbg_u2 = nc.dram_tensor("dbg_u2", [128, 512], mybir.dt.bfloat16, kind="ExternalOutput")
        dbg_u = nc.dram_tensor("dbg_u", [128, 8], mybir.dt.bfloat16, kind="ExternalOutput")
        dbg_zf = nc.dram_tensor("dbg_zf", [64, 8], F32, kind="ExternalOutput")

    with tile.TileContext(nc) as tc:
        cpool = tc.tile_pool(name="consts", bufs=1)
        cp = cpool.__enter__()

        def load_const(dram, shape, dt):
            t = cp.tile(shape, dt, tag=f"c_{dram.name}")
            nc.gpsimd.dma_start(t[:], dram[:])
            return t

        w1col_s = load_const(w1col, [25, 32], BF16)
        c1b_s = load_const(c1b, [32, 1], F32)
        w2taps_s = load_const(w2taps, [32, 288], BF16)
        c2b_s = load_const(c2b, [32, 1], F32)
        a1w_s = load_const(a1w, [32, 4], F32)
        a1b_s = load_const(a1b, [4, 1], F32)
        a2w_s = load_const(a2w, [4, 32], F32)
        a2b_s = load_const(a2b, [32, 1], F32)
        HT_s = load_const(HTd, [64, NQ], F32)
        iwT_s = load_const(iwT, [128, 256], F32)
        ib_s = load_const(ibd, [64, 1], F32)
        w1T_s = load_const(w1T, [64, 128], BF16)
        f1b_s = load_const(f1bd, [128, 1], F32)
        b2r_s = load_const(b2r, [128, 256], F32)
        owT_s = load_const(owT, [64, 2], F32)
        ob_s = load_const(obd, [2, 1], F32)
        idm_s = load_const(idmd, [32, 32], F32)
        pooled = cp.tile([32, 8192], F32)
        pooled_r = pooled[:].rearrange("p (i hp w) -> p i hp w", i=8, hp=16, w=64)

        # ---------------- CNN ----------------
        if phase >= 1:
          with tc.tile_pool(name="cnn", bufs=1) as cnn, \
             tc.tile_pool(name="cnn2", bufs=2) as cnn2, \
             tc.tile_pool(name="cnnps", bufs=2, space="PSUM") as cnnps:
            c1pad = cnn.tile([32, 8 * 34 * 130], BF16)
            nc.gpsimd.memset(c1pad[:], 0.0)
            c1pad_r = c1pad[:].rearrange("p (i h w) -> p i h w", i=8, h=34, w=130)

            # conv1, processed in 4 chunks of 8 output rows
            for hc in range(4):
                h0 = hc * 8
                imcol = cnn2.tile([25, 8192], BF16, tag="imcol")
                for dy in range(5):
                    src = _ap(x_pad[:], (h0 + dy) * 1056,
                              [(1, 5), (1056, 8), (132, 8), (1, 128)])
                    nc.gpsimd.dma_start(imcol[dy * 5:(dy + 1) * 5, :], src)
                for c in range(16):
                    h = h0 + c // 2
                    ihalf = c % 2
                    ps = cnnps.tile([32, 512], F32, tag="c1")
                    nc.tensor.matmul(ps[:], w1col_s[:], imcol[:, c * 512:(c + 1) * 512],
                                     start=True, stop=True)
                    dest = c1pad_r[:, 4 * ihalf:4 * ihalf + 4, 1 + h, 1:129]
                    nc.scalar.activation(dest, ps[:].rearrange("p (i w) -> p i w", i=4),
                                         AF.Relu, bias=c1b_s[:, 0:1])

            # conv2 (tap-accumulated) + relu + maxpool, per image / 4-row chunk
            for img in range(8):
                for hc in range(8):
                    h0 = hc * 4
                    ps2 = cnnps.tile([32, 512], F32, tag="c2")
                    for tap in range(9):
                        dy, dx = tap // 3, tap % 3
                        rhs = c1pad_r[:, img, h0 + dy:h0 + dy + 4, dx:dx + 128]
                        nc.tensor.matmul(ps2[:], w2taps_s[:, tap * 32:(tap + 1) * 32],
                                         rhs, start=(tap == 0), stop=(tap == 8))
                    c2c = cnn2.tile([32, 512], F32, tag="c2out")
                    nc.scalar.activation(c2c[:], ps2[:], AF.Relu, bias=c2b_s[:, 0:1])
                    c2r = c2c[:].rearrange("p (h a w b) -> p h a w b", h=2, a=2, w=64, b=2)
                    t1 = cnn2.tile([32, 128], F32, tag="pa")
                    t1r = t1[:].rearrange("p (h w) -> p h w", h=2)
                    t2 = cnn2.tile([32, 128], F32, tag="pb")
                    t2r = t2[:].rearrange("p (h w) -> p h w", h=2)
                    nc.vector.tensor_tensor(t1r, c2r[:, :, 0, :, 0], c2r[:, :, 0, :, 1], op=ALU.max)
                    nc.vector.tensor_tensor(t2r, c2r[:, :, 1, :, 0], c2r[:, :, 1, :, 1], op=ALU.max)
                    dest = pooled_r[:, img, h0 // 2:h0 // 2 + 2, :]
                    nc.vector.tensor_tensor(dest, t1r, t2r, op=ALU.max)

        # ---------------- attention ----------------
        if phase >= 2:
          with tc.tile_pool(name="att", bufs=1) as att, \
             tc.tile_pool(name="attps", bufs=1, space="PSUM") as attps:
            satt = att.tile([32, 8], F32)
            nc.vector.tensor_reduce(satt[:], pooled[:].rearrange("p (i f) -> p i f", i=8),
                                    axis=mybir.AxisListType.X, op=ALU.add)
            a1ps = attps.tile([4, 8], F32, tag="a1")
            nc.tensor.matmul(a1ps[:], a1w_s[:], satt[:], start=True, stop=True)
            att1 = att.tile([4, 8], F32)
            nc.scalar.activation(att1[:], a1ps[:], AF.Relu, bias=a1b_s[:, 0:1])
            a2ps = attps.tile([32, 8], F32, tag="a2")
            nc.tensor.matmul(a2ps[:], a2w_s[:], att1[:], start=True, stop=True)
            attw = att.tile([32, 8], F32)
            nc.scalar.activation(attw[:], a2ps[:], AF.Sigmoid, bias=a2b_s[:, 0:1])
            nc.vector.tensor_tensor(
                pooled[:].rearrange("p (i f) -> p i f", i=8),
                pooled[:].rearrange("p (i f) -> p i f", i=8),
                attw[:].unsqueeze(-1).broadcast_to((32, 8, 1024)),
                op=ALU.mult)

        # ---------------- spline/dX table + z0 + ODE ----------------
        if phase >= 3:
          with tc.tile_pool(name="ode", bufs=1) as ode, \
             tc.tile_pool(name="seqp", bufs=2) as seqp, \
             tc.tile_pool(name="stg", bufs=2) as stg, \
             tc.tile_pool(name="u2p", bufs=5) as u2p:

            w2sb = ode.tile([128, 32768], BF16)
            for ch in range(8):
                nc.gpsimd.dma_start(w2sb[:, ch * 4096:(ch + 1) * 4096],
                                    w2T[:, ch * 4096:(ch + 1) * 4096])
            dxtab = ode.tile([128, NQ * 32], BF16)   # [c, (q, dt, b)]
            dxtab_r = dxtab[:].rearrange("p (q c b) -> p q c b", q=NQ, c=4, b=8)

            p2T = ode.tile([64, 8 * 512], F32)   # seq, [w][img][oc*16+hp]
            p2T_r = p2T[:].rearrange("w (i o h) -> w i o h", i=8, o=32, h=16)
            with tc.tile_pool(name="dxps", bufs=2, space="PSUM") as dxps:
                for img in range(8):
                    for hp in range(16):
                        tp = dxps.tile([64, 32], F32, tag="tp")
                        nc.tensor.transpose(tp[:], pooled_r[:, img, hp, :], idm_s[:, :])
                        nc.scalar.copy(p2T_r[:, img, :, hp], tp[:])
                for b in range(BPC):
                    for ct in range(4):
                        dps = dxps.tile([128, NQ], F32, tag="dx")
                        nc.tensor.matmul(dps[:], p2T[:, b * 512 + ct * 128:b * 512 + (ct + 1) * 128],
                                         HT_s[:], start=True, stop=True)
                        nc.scalar.copy(dxtab_r[:, :, ct, b], dps[:])
                s0 = ode.tile([128, 32], F32)
                for b in range(BPC):
                    for ct in range(4):
                        sp = dxps.tile([128, 1], F32, tag="s0p")
                        nc.tensor.transpose(
                            sp[:], p2T[0:1, b * 512 + ct * 128:b * 512 + (ct + 1) * 128],
                            idm_s[0:1, 0:1])
                        nc.scalar.copy(s0[:, ct * 8 + b:ct * 8 + b + 1], sp[:])

            with tc.tile_pool(name="odeps", bufs=1, space="PSUM") as odeps, \
                 tc.tile_pool(name="mm2ps", bufs=5, space="PSUM") as mm2ps:
                z0ps = odeps.tile([64, 8], F32, tag="vfA2")
                for ct in range(4):
                    nc.tensor.matmul(z0ps[:], iwT_s[:, ct * 64:(ct + 1) * 64],
                                     s0[:, ct * 8:(ct + 1) * 8],
                                     start=(ct == 0), stop=(ct == 3))
                z_sb = ode.tile([64, 8], F32)   # state, zT layout [h, b]
                nc.scalar.activation(z_sb[:], z0ps[:], AF.Identity, bias=ib_s[:, 0:1])
                if debug_dump:
                    nc.gpsimd.dma_start(dbg_pooled[:], pooled[:])
                    nc.gpsimd.dma_start(dbg_p2T[:], p2T[:])
                    nc.gpsimd.dma_start(dbg_s0[:], s0[:])
                    nc.gpsimd.dma_start(dbg_dx[:], dxtab[:])
                    nc.gpsimd.dma_start(dbg_z0[:], z_sb[:])
                    kdmp = ode.tile([64, 32], F32)
                    u2dmp = ode.tile([128, 512], BF16)
                    udmp = ode.tile([128, 8], BF16)

                zero1 = ode.tile([128, 1], F32)
                nc.gpsimd.memset(zero1[:], 0.0)
                z_bf = ode.tile([64, 8], BF16)
                nc.vector.tensor_copy(z_bf[:], z_sb[:])

                def stt(dst, a, scal, b):
                    nc.vector.scalar_tensor_tensor(dst[:], a[:], scal, b[:],
                                                   op0=ALU.mult, op1=ALU.add)

                def vf_stage(zarg_bf, dx_ap_fn, vfA, vfB):
                    """One vf eval: mm1 -> relu -> mm2 x4 (+tanh) -> contraction.
                    dx_ap_fn(ct, b) -> [128,1] AP of dX column."""
                    u2s = []

                    def emit_pre():
                        tiles = []
                        for dt in range(4):
                            mps = mm2ps.tile([128, 512], F32, tag="mm2")
                            nc.vector.tensor_copy(
                                mps[:].rearrange("p (h b) -> p h b", h=64),
                                b2r_s[:, dt * 64:(dt + 1) * 64].unsqueeze(-1)
                                     .broadcast_to((128, 64, 8)))
                            tiles.append(mps)
                        return tiles

                    mm2tiles = emit_pre()
                    ups = odeps.tile([128, 8], F32, tag="u")
                    nc.tensor.matmul(ups[:], w1T_s[:], zarg_bf[:], start=True,
                                     stop=True)
                    ubf = stg.tile([128, 8], BF16, tag="ubf")
                    nc.vector.scalar_tensor_tensor(
                        ubf[:], ups[:], f1b_s[:, 0:1],
                        zero1[:].broadcast_to((128, 8)),
                        op0=ALU.add, op1=ALU.max)

                    def emit_mm2(dt):
                        mps = mm2tiles[dt]
                        for h in range(64):
                            j = h * 4 + dt
                            nc.tensor.matmul(mps[:, h * 8:(h + 1) * 8],
                                             w2sb[:, j * 128:(j + 1) * 128],
                                             ubf[:], start=False, stop=True,
                                             skip_group_check=True)
                        u2d = u2p.tile([128, 512], BF16, tag="u2")
                        nc.scalar.activation(u2d[:], mps[:], AF.Tanh)
                        u2s.append(u2d[:].rearrange("p (h b) -> p h b", h=64))

                    def emit_ctr(vt, dts):
                        for b in range(BPC):
                            for i, dt in enumerate(dts):
                                nc.tensor.matmul(vt[:, b:b + 1], u2s[dt][:, :, b],
                                                 dx_ap_fn(dt, b),
                                                 start=(i == 0),
                                                 stop=(i == len(dts) - 1),
                                                 skip_group_check=True)

                    for dt in range(3):
                        emit_mm2(dt)
                    emit_ctr(vfA, (0, 1, 2))
                    emit_mm2(3)
                    emit_ctr(vfB, (3,))

                # k tiles (f32, sbuf). k1 carries across steps (FSAL).
                kt = {j: ode.tile([64, 8], F32, tag=f"k{j}", name=f"kt{j}")
                      for j in range(1, 6)}
                part = {s: ode.tile([64, 8], F32, tag=f"part{s}", name=f"part{s}")
                        for s in range(3, 8)}
                vfA = odeps.tile([64, 8], F32, tag="vfA2")
                vfB = odeps.tile([64, 8], F32, tag="vfB2")

                def kmerge(dst):
                    # dst = vfA + vfB without a dual-PSUM-read instruction
                    nc.vector.tensor_copy(dst[:], vfA[:])
                    nc.vector.tensor_tensor(dst[:], dst[:], vfB[:], op=ALU.add)

                # ---- k1 = vf(0, z0), table group 0 (static offset) ----
                vf_stage(z_bf,
                         lambda ct, b: dxtab[:, ct * 8 + b:ct * 8 + b + 1],
                         vfA, vfB)
                kmerge(kt[1])

                with tc.For_i(0, nsteps) as it:
                    dxs = stg.tile([128, 160], BF16, tag="dxs")
                    idx = (it * 0 if timing_mode else it * 160) + 32
                    nc.vector.tensor_copy(dxs[:], dxtab[:, bass.ds(idx, 160)])

                    def dxg(g):
                        return lambda ct, b: dxs[:, g * 32 + ct * 8 + b:
                                                 g * 32 + ct * 8 + b + 1]

                    # partial args: part[s] = z + a_{s,1}*k1 (b-row for s=7)
                    for s in range(3, 7):
                        stt(part[s], kt[1], DP5_A[s][0], z_sb)
                    stt(part[7], kt[1], DP5_B[0], z_sb)

                    # stage 2
                    zarg = stg.tile([64, 8], BF16, tag="zarg")
                    stt(zarg, kt[1], DP5_A[2][0], z_sb)
                    vf_stage(zarg, dxg(0), vfA, vfB)
                    kmerge(kt[2])
                    stt(part[4], kt[2], DP5_A[4][1], part[4])
                    stt(part[5], kt[2], DP5_A[5][1], part[5])
                    stt(part[6], kt[2], DP5_A[6][1], part[6])
                    # stage 3
                    zarg = stg.tile([64, 8], BF16, tag="zarg")
                    stt(zarg, kt[2], DP5_A[3][1], part[3])
                    vf_stage(zarg, dxg(1), vfA, vfB)
                    kmerge(kt[3])
                    stt(part[5], kt[3], DP5_A[5][2], part[5])
                    stt(part[6], kt[3], DP5_A[6][2], part[6])
                    stt(part[7], kt[3], DP5_B[2], part[7])
                    # stage 4
                    zarg = stg.tile([64, 8], BF16, tag="zarg")
                    stt(zarg, kt[3], DP5_A[4][2], part[4])
                    vf_stage(zarg, dxg(2), vfA, vfB)
                    kmerge(kt[4])
                    stt(part[6], kt[4], DP5_A[6][3], part[6])
                    stt(part[7], kt[4], DP5_B[3], part[7])
                    # stage 5
                    zarg = stg.tile([64, 8], BF16, tag="zarg")
                    stt(zarg, kt[4], DP5_A[5][3], part[5])
                    vf_stage(zarg, dxg(3), vfA, vfB)
                    kmerge(kt[5])
                    stt(part[7], kt[5], DP5_B[4], part[7])
                    # stage 6 (k6 only feeds z_{n+1}; fold it in directly)
                    zarg = stg.tile([64, 8], BF16, tag="zarg")
                    stt(zarg, kt[5], DP5_A[6][4], part[6])
                    vf_stage(zarg, dxg(4), vfA, vfB)
                    ztmp = stg.tile([64, 8], F32, tag="ztmp")
                    stt(ztmp, vfA, DP5_B[5], part[7])
                    stt(z_bf, vfB, DP5_B[5], ztmp)  # z_{n+1} bf16 gates stage 7
                    stt(z_sb, vfB, DP5_B[5], ztmp)
                    # stage 7 = vf(t+1, z_{n+1}) -> next step's k1 (FSAL)
                    vf_stage(z_bf, dxg(4), vfA, vfB)
                    kmerge(kt[1])

                # ---------------- output head ----------------
                if debug_dump:
                    nc.gpsimd.dma_start(dbg_zf[:], z_sb[:])
                    nc.gpsimd.dma_start(dbg_k[:], kdmp[:])
                    nc.gpsimd.dma_start(dbg_u2[:], u2dmp[:])
                    nc.gpsimd.dma_start(dbg_u[:], udmp[:])
                ops_ = odeps.tile([2, 8], F32, tag="u")
                nc.tensor.matmul(ops_[:], owT_s[:, :], z_sb[:], start=True, stop=True)
                osb = ode.tile([2, 8], F32)
                nc.scalar.activation(osb[:], ops_[:], AF.Identity, bias=ob_s[:, 0:1])
                dst = _ap(out_d[:], 0, [(1, 2), (2, 8)])
                nc.gpsimd.dma_start(dst, osb[:])

        cpool.__exit__(None, None, None)

    nc.compile()
    return nc


_CACHE = {}


class _Runner:
    """Persistent PJRT executor: jit+shard_map built once, weights resident
    on device across calls (only x + tiny donated output buffers move)."""

    def __init__(self):
        import jax
        from jax.sharding import Mesh, PartitionSpec, NamedSharding
        from jax.experimental.shard_map import shard_map
        from concourse import bass2jax as b2j

        b2j.install_neuronx_cc_hook()
        nc = _build()
        self.nc = nc
        self.dbg_name = None
        if nc.dbg_addr is not None:
            if nc.dbg_callbacks:
                raise RuntimeError("dbg_callbacks unsupported in cached runner")
            self.dbg_name = nc.dbg_addr.name
        partition_name = (nc.partition_id_tensor.name
                          if nc.partition_id_tensor else None)
        in_names, out_names, out_avals, zero_shapes = [], [], [], []
        for alloc in nc.m.functions[0].allocations:
            if not isinstance(alloc, mybir.MemoryLocationSet):
                continue
            name = alloc.memorylocations[0].name
            if alloc.kind == "ExternalInput":
                if name != partition_name:
                    in_names.append(name)
            elif alloc.kind == "ExternalOutput":
                shape = tuple(alloc.tensor_shape)
                dtype = mybir.dt.np(alloc.dtype)
                out_names.append(name)
                out_avals.append(jax.core.ShapedArray(shape, dtype))
                zero_shapes.append((shape, dtype))
        self.param_names = list(in_names)
        self.out_names = out_names
        self.zero_shapes = zero_shapes
        n_params = len(in_names)
        n_outs = len(out_names)
        all_in_names = in_names + out_names
        if partition_name is not None:
            all_in_names.append(partition_name)

        def _body(*args):
            operands = list(args)
            if partition_name is not None:
                operands.append(b2j.partition_id_tensor())
            outs = b2j._bass_exec_p.bind(
                *operands,
                out_avals=tuple(out_avals),
                in_names=tuple(all_in_names),
                out_names=tuple(out_names),
                lowering_input_output_aliases=(),
                sim_require_finite=True,
                sim_require_nnan=True,
                nc=nc,
            )
            return tuple(outs)

        devices = jax.devices()[:N_CORES]
        assert len(devices) == N_CORES
        self.mesh = Mesh(np.asarray(devices), ("core",))
        self.sharding = NamedSharding(self.mesh, PartitionSpec("core"))
        in_specs = (PartitionSpec("core"),) * (n_params + n_outs)
        out_specs = (PartitionSpec("core"),) * n_outs
        self.sharded = jax.jit(
            shard_map(_body, mesh=self.mesh, in_specs=in_specs,
                      out_specs=out_specs, check_rep=False),
            donate_argnums=tuple(range(n_params, n_params + n_outs)),
            keep_unused=True,
        )
        self.wkey = None
        self.static_dev = None
        self._device_put = jax.device_put

    def prep_weights(self, inputs):
        key = tuple(id(inputs[k]) for k in sorted(inputs) if k != "x")
        if key == self.wkey:
            return
        sh = _shared_inputs(inputs)
        if self.dbg_name is not None:
            sh[self.dbg_name] = np.zeros((1, 2), np.uint32)
        dev = {}
        for name in self.param_names:
            if name == "x_pad":
                continue
            a = sh[name]
            g = np.broadcast_to(a[None], (N_CORES,) + a.shape).reshape(
                (N_CORES * a.shape[0],) + a.shape[1:])
            dev[name] = self._device_put(np.ascontiguousarray(g), self.sharding)
        for v in dev.values():
            v.block_until_ready()
        self.static_dev = dev
        self.wkey = key

    def __call__(self, inputs):
        self.prep_weights(inputs)
        xg = _x_global(inputs["x"])
        args = [xg if n == "x_pad" else self.static_dev[n]
                for n in self.param_names]
        zouts = [np.zeros((N_CORES * s[0],) + tuple(s[1:]), d)
                 for (s, d) in self.zero_shapes]
        outs = self.sharded(*args, *zouts)
        oi = self.out_names.index("out")
        return np.asarray(outs[oi])  # [64, 2]


def _shared_inputs(inputs):
    bf = ml_dtypes.bfloat16
    c1w = np.asarray(inputs["conv1_w"], np.float32)
    c2w = np.asarray(inputs["conv2_w"], np.float32)
    sh = {
        "w1col": np.ascontiguousarray(c1w.reshape(32, 25).T.astype(bf)),
        "c1b": np.asarray(inputs["conv1_b"], np.float32).reshape(32, 1),
        "w2taps": np.ascontiguousarray(
            np.concatenate([c2w[:, :, dy, dx].T for dy in range(3) for dx in range(3)],
                           axis=1).astype(bf)),
        "c2b": np.asarray(inputs["conv2_b"], np.float32).reshape(32, 1),
        "a1w": np.ascontiguousarray(
            (np.asarray(inputs["att_fc1_w"], np.float32) / 1024.0).T),
        "a1b": np.asarray(inputs["att_fc1_b"], np.float32).reshape(4, 1),
        "a2w": np.ascontiguousarray(np.asarray(inputs["att_fc2_w"], np.float32).T),
        "a2b": np.asarray(inputs["att_fc2_b"], np.float32).reshape(32, 1),
        "HT": np.ascontiguousarray(_make_H().T),
        "iwT": np.ascontiguousarray(
            np.asarray(inputs["initial_w"], np.float32).T.reshape(4, 128, 64)
              .transpose(1, 0, 2).reshape(128, 256)),
        "ib": np.asarray(inputs["initial_b"], np.float32).reshape(64, 1),
        "w1T": np.ascontiguousarray(np.asarray(inputs["f1_w"], np.float32).T.astype(bf)),
        "f1b": np.asarray(inputs["f1_b"], np.float32).reshape(128, 1),
        "w2T": np.ascontiguousarray(np.asarray(inputs["f2_w"], np.float32).T.astype(bf)),
        "b2r": np.ascontiguousarray(
            np.asarray(inputs["f2_b"], np.float32).reshape(64, 4, 128)
              .transpose(2, 1, 0).reshape(128, 256)),
        "owT": np.ascontiguousarray(np.asarray(inputs["out_w"], np.float32).T),
        "ob": np.asarray(inputs["out_b"], np.float32).reshape(2, 1),
        "idm": np.eye(32, dtype=np.float32),
    }
    return sh


def _x_shard(x, core):
    bf = ml_dtypes.bfloat16
    xs = np.asarray(x, np.float32)[core * BPC:(core + 1) * BPC, 0]  # [8,32,128]
    xp = np.zeros((36, 8, 132), np.float32)
    xp[2:34, :, 2:130] = xs.transpose(1, 0, 2)
    return np.ascontiguousarray(xp.reshape(36, 8 * 132).astype(bf))


def _x_global(x):
    """All 8 core shards stacked on axis 0: [8*36, 8*132] bf16."""
    bf = ml_dtypes.bfloat16
    xs = np.asarray(x, np.float32)[:, 0].reshape(N_CORES, BPC, 32, 128)
    xp = np.zeros((N_CORES, 36, BPC, 132), np.float32)
    xp[:, 2:34, :, 2:130] = xs.transpose(0, 2, 1, 3)
    return xp.reshape(N_CORES * 36, BPC * 132).astype(bf)


def kernel(**inputs):
    if "runner" not in _CACHE:
        _CACHE["runner"] = _Runner()
    return _CACHE["runner"](inputs)


if __name__ == "__main__":
    rng = np.random.default_rng(0)
    ins = {
        "x": rng.standard_normal((64, 1, 32, 128)).astype(np.float32),
        "conv1_w": (rng.standard_normal((32, 1, 5, 5)) * 0.05).astype(np.float32),
        "conv1_b": np.zeros(32, np.float32),
        "conv2_w": (rng.standard_normal((32, 32, 3, 3)) * 0.05).astype(np.float32),
        "conv2_b": np.zeros(32, np.float32),
        "att_fc1_w": (rng.standard_normal((4, 32)) * 0.05).astype(np.float32),
        "att_fc1_b": np.zeros(4, np.float32),
        "att_fc2_w": (rng.standard_normal((32, 4)) * 0.05).astype(np.float32),
        "att_fc2_b": np.zeros(32, np.float32),
        "initial_w": (rng.standard_normal((64, 512)) * 0.05).astype(np.float32),
        "initial_b": np.zeros(64, np.float32),
        "f1_w": (rng.standard_normal((128, 64)) * 0.05).astype(np.float32),
        "f1_b": np.zeros(128, np.float32),
        "f2_w": (rng.standard_normal((512 * 64, 128)) * 0.05).astype(np.float32),
        "f2_b": np.zeros(512 * 64, np.float32),
        "out_w": (rng.standard_normal((2, 64)) * 0.05).astype(np.float32),
        "out_b": np.zeros(2, np.float32),
    }
    out = kernel(**ins)
    print("kernel output", out.shape, out[:2])

